# revision 1
# baseline (speedup 1.0000x reference)
"""Trainium2 Bass kernel for nn_Block_16441134809284 (sparse_attention block).

Self-contained: token-parallel over 8 NeuronCores (2 batches x 4 slices of 512
tokens). Each core computes its 512 output tokens end-to-end; KV for the
sliding window is recomputed per core from a zero-padded token window, so no
collectives are needed. All heavy math runs on the TensorEngine in float32r
(fp32 storage, FP22 multiply) at full bf16-rate.

Host-side prep (numpy): mask analysis -> per-core kv window + additive bias
tensors; rope sin/cos tables; weight re-layout; fold (1+pre_*_scale) into the
following projection weights. Softmax is computed without max-subtraction
(logits are soft-capped to +-50, so exp() cannot overflow); masking is an
additive bias of 0 or K_MASK/SOFT_CAP applied inside exp(SOFT_CAP * (tanh(l /
SOFT_CAP) + bias)).
"""
import sys

for _p in ("/opt/trn_rl_repo", "/root/.axon_site/_ro/trn_rl_repo"):
    if _p not in sys.path:
        sys.path.insert(0, _p)

import numpy as np

K_MASK = -2.3819763e+38
SOFT_CAP = 50.0
WINDOW = 1024
ROPE_BASE = 10000.0

B, T, D, N, KH, H, F = 2, 2048, 2048, 8, 4, 256, 8192
CACHE = 4096
N_CORES = 8
SLICES = N_CORES // B
TQ = T // SLICES          # 512
EPS = 1e-6
CH = 128                  # chunk (partition) size
DCH = D // CH             # 16
HCH = H // CH             # 2
SEG_CH = 4                # kv chunks per attention segment (512 tokens)
SEG = SEG_CH * CH         # 512
FPG = 8                   # F-chunks per FFN group
FGROUPS = F // CH // FPG  # 4
G = N // KH               # 2 query heads per kv head


# ----------------------------------------------------------------------------
# host-side planning (validated against the reference in numpy, see proto.py)
# ----------------------------------------------------------------------------

def _plan(inputs):
    attn_mask = np.asarray(inputs['attn_mask'])
    seg_pos = np.asarray(inputs['segment_pos']).astype(np.int64)
    cache_pos_in = np.asarray(inputs['cache_positions']).astype(np.int64)
    end_index = np.asarray(inputs['end_index']).astype(np.int64)
    x = np.asarray(inputs['x'], dtype=np.float32)

    slot_of_tok = (end_index[:, None] + np.arange(T)[None, :]) % CACHE
    old_slots = (end_index[:, None] + T + np.arange(CACHE - T)[None, :]) % CACHE

    cache_pos = cache_pos_in.copy()
    bidx = np.arange(B)[:, None]
    cache_pos[bidx, slot_of_tok] = seg_pos
    sliding = (cache_pos[:, None, :] > seg_pos[:, :, None] - WINDOW) & \
              (cache_pos[:, None, :] < seg_pos[:, :, None] + WINDOW)
    mask = attn_mask & sliding                      # [B, T(query), S(slot)]

    mask_tok = np.take_along_axis(mask, slot_of_tok[:, None, :], axis=2)
    mask_old = np.take_along_axis(mask, old_slots[:, None, :], axis=2)

    L_need = R_need = 0
    cache_chunks_needed = 0
    n_old = CACHE - T
    for b in range(B):
        for s in range(SLICES):
            t0 = s * TQ
            used = mask_tok[b, t0:t0 + TQ].any(axis=0)
            cidx = np.nonzero(used.reshape(T // CH, CH).any(axis=1))[0]
            if len(cidx):
                L_need = max(L_need, t0 // CH - int(cidx[0]))
                R_need = max(R_need, int(cidx[-1]) + 1 - (t0 + TQ) // CH)
            used_o = mask_old[b, t0:t0 + TQ].any(axis=0)
            co = np.nonzero(used_o.reshape(n_old // CH, CH).any(axis=1))[0]
            cache_chunks_needed = max(cache_chunks_needed, len(co))

    # own-token offset must be SEG-aligned so q can be built from whole blocks
    L_need = ((max(L_need, 0) + SEG_CH - 1) // SEG_CH) * SEG_CH
    R_need = max(R_need, 0)
    W = L_need + TQ // CH + R_need
    W = ((W + SEG_CH - 1) // SEG_CH) * SEG_CH
    OWN_OFF = L_need * CH
    KV_TOK = W * CH
    EXTRA = ((cache_chunks_needed + SEG_CH - 1) // SEG_CH) * SEG_CH \
        if cache_chunks_needed else 0

    per_core = []
    frac = 2.0 * np.arange(H // 2, dtype=np.float32) / np.float32(H)
    timescale = np.float32(ROPE_BASE) ** frac
    for c in range(N_CORES):
        b, s = divmod(c, SLICES)
        t0 = s * TQ
        toks = np.arange(t0 - OWN_OFF, t0 - OWN_OFF + KV_TOK)
        valid = (toks >= 0) & (toks < T)
        tv = np.clip(toks, 0, T - 1)

        xw = np.where(valid[:, None], x[b, tv], 0.0).astype(np.float32)
        x_T = np.ascontiguousarray(xw.T)            # [D, KV_TOK]

        pos = np.where(valid, seg_pos[b, tv], 0).astype(np.float32)
        ang = pos[None, :] / timescale[:, None]
        sin_t = np.sin(ang).astype(np.float32)      # [128, KV_TOK]
        cos_t = np.cos(ang).astype(np.float32)

        mb = mask_tok[b, t0:t0 + TQ][:, tv] & valid[None, :]
        bias_c = np.where(mb.T, 1.0, 0.0).astype(np.float32)
        bias_c = np.ascontiguousarray(bias_c)       # [KV_TOK, TQ]

        entry = dict(b=b, t0=t0, x_T=x_T, sin=sin_t, cos=cos_t, bias=bias_c)

        if EXTRA:
            n_ex = EXTRA * CH
            mo = mask_old[b, t0:t0 + TQ]
            used_o = mo.any(axis=0)
            order = np.argsort(~used_o, kind='stable')
            sel = order[:n_ex]
            ck = np.asarray(inputs['cache_k'], dtype=np.float32)[b][old_slots[b][sel]]
            cv = np.asarray(inputs['cache_v'], dtype=np.float32)[b][old_slots[b][sel]]
            entry['kc_T'] = np.ascontiguousarray(ck.transpose(1, 2, 0))  # [KH, H, n_ex]
            entry['vc'] = np.ascontiguousarray(cv.transpose(1, 0, 2))    # [KH, n_ex, H]
            entry['bias_cache'] = np.ascontiguousarray(
                np.where(mo[:, sel].T, 1.0, 0.0).astype(np.float32))
        per_core.append(entry)

    return dict(W=W, OWN_OFF=OWN_OFF, KV_TOK=KV_TOK, EXTRA=EXTRA,
                per_core=per_core)


def _prep_weights(inputs):
    w_kv = np.asarray(inputs['w_kv'], dtype=np.float32)
    pre_attn = (1.0 + np.asarray(inputs['pre_attn_scale'], dtype=np.float32))
    pre_ffw = (1.0 + np.asarray(inputs['pre_ffw_scale'], dtype=np.float32))
    w_g = np.asarray(inputs['w_gating'], dtype=np.float32)        # [2, F, D]
    w_g_T = np.ascontiguousarray(w_g.transpose(0, 2, 1))          # [2, D, F]
    w_g_T = w_g_T * pre_ffw[None, :, None]                        # fold pre_ffw
    return dict(
        w_q=np.ascontiguousarray(np.asarray(inputs['w_q'], dtype=np.float32)
                                 * pre_attn[None, :, None]),      # [N, D, H]
        w_k=np.ascontiguousarray(w_kv[0] * pre_attn[None, :, None]),
        w_v=np.ascontiguousarray(w_kv[1] * pre_attn[None, :, None]),
        w_av=np.ascontiguousarray(np.asarray(inputs['w_attn_vec'], dtype=np.float32)),
        w_g_T=np.ascontiguousarray(w_g_T),
        w_lin=np.ascontiguousarray(np.asarray(inputs['w_linear'], dtype=np.float32)),
        q_scale=np.ascontiguousarray(
            (1.0 + np.asarray(inputs['q_norm_scale'], dtype=np.float32))
            .reshape(HCH, CH).T),                                 # [128, 2]
        k_scale=np.ascontiguousarray(
            (1.0 + np.asarray(inputs['k_norm_scale'], dtype=np.float32))
            .reshape(HCH, CH).T),
        post_attn=np.ascontiguousarray(
            (1.0 + np.asarray(inputs['post_attn_scale'], dtype=np.float32))
            .reshape(DCH, CH).T),                                 # [128, 16]
        post_ffw=np.ascontiguousarray(
            (1.0 + np.asarray(inputs['post_ffw_scale'], dtype=np.float32))
            .reshape(DCH, CH).T),
        skip=float(np.asarray(inputs['skip_scale']).reshape(-1)[0]),
    )


# ----------------------------------------------------------------------------
# device kernel builder
# ----------------------------------------------------------------------------

def _build_nc(W, OWN_OFF, EXTRA, skip):
    import concourse.bass as bass  # noqa: F401
    import concourse.tile as tile
    from concourse import mybir, bacc
    from contextlib import ExitStack

    F32 = mybir.dt.float32
    F32R = mybir.dt.float32r
    AF = mybir.ActivationFunctionType
    OP = mybir.AluOpType

    KV_TOK = W * CH
    NSEG = W // SEG_CH
    NSEG_X = EXTRA // SEG_CH if EXTRA else 0

    nc = bacc.Bacc()
    d_x = nc.declare_dram_parameter("x_T", [D, KV_TOK], F32R, isOutput=False)
    d_sin = nc.declare_dram_parameter("sin_t", [CH, KV_TOK], F32, isOutput=False)
    d_cos = nc.declare_dram_parameter("cos_t", [CH, KV_TOK], F32, isOutput=False)
    d_bias = nc.declare_dram_parameter("bias", [KV_TOK, TQ], F32, isOutput=False)
    d_wq = nc.declare_dram_parameter("w_q", [N, D, H], F32R, isOutput=False)
    d_wk = nc.declare_dram_parameter("w_k", [KH, D, H], F32R, isOutput=False)
    d_wv = nc.declare_dram_parameter("w_v", [KH, D, H], F32R, isOutput=False)
    d_wav = nc.declare_dram_parameter("w_av", [N, H, D], F32R, isOutput=False)
    d_wg = nc.declare_dram_parameter("w_g_T", [2, D, F], F32R, isOutput=False)
    d_wl = nc.declare_dram_parameter("w_lin", [F, D], F32R, isOutput=False)
    d_qs = nc.declare_dram_parameter("q_scale", [CH, HCH], F32, isOutput=False)
    d_ks = nc.declare_dram_parameter("k_scale", [CH, HCH], F32, isOutput=False)
    d_pa = nc.declare_dram_parameter("post_attn", [CH, DCH], F32, isOutput=False)
    d_pf = nc.declare_dram_parameter("post_ffw", [CH, DCH], F32, isOutput=False)
    d_ones = nc.declare_dram_parameter("onesd", [CH, CH], F32R, isOutput=False)
    if EXTRA:
        d_kc = nc.declare_dram_parameter("kc_T", [KH, H, EXTRA * CH], F32R, isOutput=False)
        d_vc = nc.declare_dram_parameter("vc", [KH, EXTRA * CH, H], F32R, isOutput=False)
        d_biasc = nc.declare_dram_parameter("bias_cache", [EXTRA * CH, TQ], F32, isOutput=False)
    d_out = nc.declare_dram_parameter("out_T", [D, TQ], F32, isOutput=True)
    d_scr = nc.dram_tensor("attn_scratch", [DCH, CH, TQ], F32)

    xr = d_x.rearrange("(dc p) t -> p dc t", p=CH)          # [128, 16, KV_TOK]
    br = d_bias.rearrange("(c p) t -> p c t", p=CH)         # [128, W, TQ]
    wgr = [d_wg[g].rearrange("(dc p) f -> p dc f", p=CH) for g in range(2)]
    wlr = d_wl.rearrange("(fc p) d2 -> p fc d2", p=CH)      # [128, 64, D]
    outr = d_out.rearrange("(dc p) t -> p dc t", p=CH)
    if EXTRA:
        bcr = d_biasc.rearrange("(c p) t -> p c t", p=CH)

    with tile.TileContext(nc) as tc, \
            nc.allow_low_precision(reason="float32r is bit-identical fp32 storage"), \
            ExitStack() as ctx:
        cpool = ctx.enter_context(tc.tile_pool(name="const", bufs=1))
        ones = cpool.tile([CH, CH], F32R)
        nc.sync.dma_start(ones[:], d_ones[:])
        eps1 = cpool.tile([1, 1], F32)
        nc.vector.memset(eps1[:], EPS)
        epsp = cpool.tile([CH, 1], F32)
        nc.vector.memset(epsp[:], EPS)
        qs_t = cpool.tile([CH, HCH], F32)
        nc.sync.dma_start(qs_t[:], d_qs[:])
        ks_t = cpool.tile([CH, HCH], F32)
        nc.sync.dma_start(ks_t[:], d_ks[:])
        pa_t = cpool.tile([CH, DCH], F32)
        nc.sync.dma_start(pa_t[:], d_pa[:])
        pf_t = cpool.tile([CH, DCH], F32)
        nc.sync.dma_start(pf_t[:], d_pf[:])
        cs_own = cpool.tile([CH, 2, TQ], F32)
        nc.sync.dma_start(cs_own[:, 0, :], d_cos[:, OWN_OFF:OWN_OFF + TQ])
        nc.sync.dma_start(cs_own[:, 1, :], d_sin[:, OWN_OFF:OWN_OFF + TQ])

        ps_lg = ctx.enter_context(tc.tile_pool(name="ps_lg", bufs=2, space="PSUM"))
        ps_kv = ctx.enter_context(tc.tile_pool(name="ps_kv", bufs=3, space="PSUM"))
        ps_O = ctx.enter_context(tc.tile_pool(name="ps_O", bufs=1, space="PSUM"))
        ps_st = ctx.enter_context(tc.tile_pool(name="ps_st", bufs=1, space="PSUM"))
        ps_bc = ctx.enter_context(tc.tile_pool(name="ps_bc", bufs=1, space="PSUM"))

        def stat_rstd(pool, src_slices, inv_n, tag):
            """ones-matmul sum-of-squares over partition chunks -> [128, n]
            broadcast of 1/sqrt(mean+eps), living in a PSUM tile."""
            n = src_slices[0].shape[-1]
            stp = ps_st.tile([1, n], F32, tag="stat", name="stp")
            for i, sl in enumerate(src_slices):
                sq = pool.tile([CH, n], F32R, tag=f"sq_{tag}", name="sq")
                nc.scalar.activation(out=sq[:], in_=sl, func=AF.Square, scale=1.0)
                nc.tensor.matmul(stp[:], ones[:, :1], sq[:],
                                 start=(i == 0), stop=(i == len(src_slices) - 1))
            std = pool.tile([1, n], F32, tag=f"std_{tag}", name="std")
            nc.scalar.activation(out=std[:], in_=stp[:], func=AF.Sqrt,
                                 bias=eps1[:], scale=inv_n)
            rstd = pool.tile([1, n], F32R, tag=f"rstd_{tag}", name="rstd")
            nc.vector.reciprocal(rstd[:], std[:])
            bcp = ps_bc.tile([CH, n], F32, tag="bc", name="bcp")
            nc.tensor.matmul(bcp[:], ones[:1, :], rstd[:], start=True, stop=True)
            bc = pool.tile([CH, n], F32, tag=f"bc_{tag}", name="bc")
            nc.scalar.activation(out=bc[:], in_=bcp[:], func=AF.Copy, scale=1.0)
            return bc

        def build_h(xpool, spool, blk_slice):
            """h = rms_norm(x) for one 512-token block, normalized in place."""
            xb = xpool.tile([CH, DCH, TQ], F32R, tag="xb", name="xb")
            nc.sync.dma_start(xb[:], xr[:, :, blk_slice])
            bc = stat_rstd(spool, [xb[:, dc, :] for dc in range(DCH)], 1.0 / D, "h")
            for dc in range(DCH):
                nc.vector.tensor_tensor(xb[:, dc, :], xb[:, dc, :], bc[:], OP.mult)
            return xb

        # ==================================================================
        # attention
        # ==================================================================
        with tc.tile_pool(name="bpool", bufs=1) as bpool, \
                tc.tile_pool(name="xbp", bufs=1) as xbp:
            qall = bpool.tile([CH, N, HCH, TQ], F32R)      # 32KB
            Oall = bpool.tile([CH, N, HCH, TQ], F32R)      # 32KB
            sums = bpool.tile([1, N, TQ], F32)             # 16KB col space

            # ---- kv segments ----
            with tc.tile_pool(name="attn1", bufs=1) as ap1, \
                    tc.tile_pool(name="attn2", bufs=1) as ap2, \
                    tc.tile_pool(name="expp", bufs=4) as epp, \
                    tc.tile_pool(name="wk_s", bufs=2) as wks:

                def attend(kh, kT_seg, v_seg, bias_seg, first):
                    for g in range(G):
                        n_head = kh * G + g
                        eps_ = []
                        for st in range(SEG_CH):
                            lg = ps_lg.tile([CH, TQ], F32, tag="mm", name="lg")
                            for hc in range(HCH):
                                nc.tensor.matmul(
                                    lg[:], kT_seg[:, hc, st * CH:(st + 1) * CH],
                                    qall[:, n_head, hc, :],
                                    start=(hc == 0), stop=(hc == HCH - 1))
                            ep = epp.tile([CH, TQ], F32R, tag="expp", name="ep")
                            nc.scalar.activation(out=ep[:], in_=lg[:],
                                                 func=AF.Tanh, scale=1.0 / SOFT_CAP)
                            nc.scalar.activation(out=ep[:], in_=ep[:],
                                                 func=AF.Exp, scale=SOFT_CAP)
                            nc.vector.tensor_tensor(ep[:], ep[:], bias_seg[:, st, :], OP.mult)
                            eps_.append(ep)
                        for hc in range(HCH):
                            op = ps_O.tile([CH, TQ], F32, tag="opsum", name="op")
                            for st in range(SEG_CH):
                                nc.tensor.matmul(
                                    op[:], v_seg[:, st, hc * CH:(hc + 1) * CH],
                                    eps_[st][:], start=(st == 0), stop=(st == SEG_CH - 1))
                            if first:
                                nc.scalar.activation(out=Oall[:, n_head, hc, :],
                                                     in_=op[:], func=AF.Copy, scale=1.0)
                            else:
                                nc.vector.tensor_tensor(Oall[:, n_head, hc, :],
                                                        Oall[:, n_head, hc, :],
                                                        op[:], OP.add)
                        sp = ps_st.tile([1, TQ], F32, tag="stat", name="sp")
                        for st in range(SEG_CH):
                            nc.tensor.matmul(sp[:], ones[:, :1], eps_[st][:],
                                             start=(st == 0), stop=(st == SEG_CH - 1))
                        if first:
                            nc.scalar.activation(out=sums[:, n_head, :], in_=sp[:],
                                                 func=AF.Copy, scale=1.0)
                        else:
                            nc.vector.tensor_tensor(sums[:, n_head, :],
                                                    sums[:, n_head, :], sp[:], OP.add)

                OWN_SEG = OWN_OFF // SEG
                seg_order = [OWN_SEG] + [s for s in range(NSEG) if s != OWN_SEG]
                for idx, seg in enumerate(seg_order):
                    ssl = slice(seg * SEG, (seg + 1) * SEG)
                    h_seg = build_h(xbp, ap2, ssl)
                    if idx == 0:
                        # q for all heads, from the own-token segment's h
                        for n_head in range(N):
                            qp = [ps_kv.tile([CH, TQ], F32, tag="mm", name=f"qp{_hc}")
                                  for _hc in range(HCH)]
                            wq_t = wks.tile([CH, DCH, H], F32R, tag="wkv", name="wq_t")
                            nc.sync.dma_start(
                                wq_t[:], d_wq[n_head].rearrange("(dc p) h2 -> p dc h2", p=CH))
                            for dc in range(DCH):
                                for hc in range(HCH):
                                    nc.tensor.matmul(qp[hc][:],
                                                     wq_t[:, dc, hc * CH:(hc + 1) * CH],
                                                     h_seg[:, dc, :],
                                                     start=(dc == 0), stop=(dc == DCH - 1))
                            bc = stat_rstd(ap2, [qp[hc][:] for hc in range(HCH)], 1.0 / H, "k")
                            qn = ap1.tile([CH, HCH, TQ], F32, tag="kseg", name="qn")
                            for hc in range(HCH):
                                nc.vector.tensor_tensor(qn[:, hc, :], qp[hc][:], bc[:], OP.mult)
                                nc.vector.tensor_scalar_mul(qn[:, hc, :], qn[:, hc, :],
                                                            qs_t[:, hc:hc + 1])
                            t0_ = ap1.tile([CH, TQ], F32, tag="kropet0", name="t0_")
                            t1_ = ap1.tile([CH, TQ], F32, tag="kropet1", name="t1_")
                            nc.vector.tensor_tensor(t0_[:], qn[:, 0, :], cs_own[:, 1, :], OP.mult)
                            nc.vector.tensor_tensor(t1_[:], qn[:, 1, :], cs_own[:, 1, :], OP.mult)
                            nc.vector.tensor_tensor(qn[:, 0, :], qn[:, 0, :], cs_own[:, 0, :], OP.mult)
                            nc.vector.tensor_tensor(qn[:, 1, :], qn[:, 1, :], cs_own[:, 0, :], OP.mult)
                            nc.vector.tensor_tensor(qall[:, n_head, 0, :], qn[:, 0, :], t1_[:], OP.subtract)
                            nc.vector.tensor_tensor(qall[:, n_head, 1, :], qn[:, 1, :], t0_[:], OP.add)
                    cseg = ap1.tile([CH, 2, SEG], F32, tag="cseg", name="cseg")
                    nc.sync.dma_start(cseg[:, 0, :], d_cos[:, ssl])
                    nc.sync.dma_start(cseg[:, 1, :], d_sin[:, ssl])
                    bias_seg = ap1.tile([CH, SEG_CH, TQ], F32, tag="biasseg", name="bias_seg")
                    nc.sync.dma_start(bias_seg[:],
                                      br[:, seg * SEG_CH:(seg + 1) * SEG_CH, :])

                    def build_kv(kh):
                        kp = [ps_kv.tile([CH, SEG], F32, tag="mm", name=f"kp{_hc}")
                              for _hc in range(HCH)]
                        wk_t = wks.tile([CH, DCH, H], F32R, tag="wkv", name="wk_t")
                        nc.sync.dma_start(
                            wk_t[:], d_wk[kh].rearrange("(dc p) h2 -> p dc h2", p=CH))
                        for dc in range(DCH):
                            for hc in range(HCH):
                                nc.tensor.matmul(kp[hc][:],
                                                 wk_t[:, dc, hc * CH:(hc + 1) * CH],
                                                 h_seg[:, dc, :],
                                                 start=(dc == 0), stop=(dc == DCH - 1))
                        bc = stat_rstd(ap2, [kp[hc][:] for hc in range(HCH)], 1.0 / H, "k")
                        kT_seg = ap1.tile([CH, HCH, SEG], F32R, tag="kseg", name="kT_seg")
                        for hc in range(HCH):
                            nc.vector.tensor_tensor(kT_seg[:, hc, :], kp[hc][:], bc[:], OP.mult)
                            nc.vector.tensor_scalar_mul(kT_seg[:, hc, :], kT_seg[:, hc, :],
                                                        ks_t[:, hc:hc + 1])
                        t0_ = ap1.tile([CH, SEG], F32, tag="kropet0", name="t0_")
                        t1_ = ap1.tile([CH, SEG], F32, tag="kropet1", name="t1_")
                        nc.vector.tensor_tensor(t0_[:], kT_seg[:, 0, :], cseg[:, 1, :], OP.mult)
                        nc.vector.tensor_tensor(t1_[:], kT_seg[:, 1, :], cseg[:, 1, :], OP.mult)
                        nc.vector.tensor_tensor(kT_seg[:, 0, :], kT_seg[:, 0, :], cseg[:, 0, :], OP.mult)
                        nc.vector.tensor_tensor(kT_seg[:, 1, :], kT_seg[:, 1, :], cseg[:, 0, :], OP.mult)
                        nc.vector.tensor_tensor(kT_seg[:, 0, :], kT_seg[:, 0, :], t1_[:], OP.subtract)
                        nc.vector.tensor_tensor(kT_seg[:, 1, :], kT_seg[:, 1, :], t0_[:], OP.add)
                        v_seg = ap1.tile([CH, SEG_CH, H], F32R, tag="vseg", name="v_seg")
                        wv_t = wks.tile([CH, DCH, H], F32R, tag="wkv", name="wv_t")
                        nc.sync.dma_start(
                            wv_t[:], d_wv[kh].rearrange("(dc p) h2 -> p dc h2", p=CH))
                        vps = [ps_kv.tile([CH, 2, H], F32, tag="mm", name=f"vps{_i}")
                               for _i in range(SEG_CH // 2)]
                        ssum4 = ap2.tile([CH, SEG_CH], F32, tag="vssum", name="ssum4")
                        for st in range(SEG_CH):
                            vp = vps[st // 2][:, st % 2, :]
                            for dc in range(DCH):
                                nc.tensor.matmul(vp,
                                                 h_seg[:, dc, st * CH:(st + 1) * CH],
                                                 wv_t[:, dc, :],
                                                 start=(dc == 0), stop=(dc == DCH - 1))
                            sqv = ap2.tile([CH, H], F32, tag="sqv", name="sqv")
                            nc.scalar.activation(out=sqv[:], in_=vp, func=AF.Square, scale=1.0)
                            nc.vector.reduce_sum(ssum4[:, st:st + 1], sqv[:],
                                                 axis=mybir.AxisListType.X)
                        nc.scalar.activation(out=ssum4[:], in_=ssum4[:], func=AF.Sqrt,
                                             bias=epsp[:], scale=1.0 / H)
                        nc.vector.reciprocal(ssum4[:], ssum4[:])
                        for st in range(SEG_CH):
                            nc.vector.tensor_scalar_mul(v_seg[:, st, :],
                                                        vps[st // 2][:, st % 2, :],
                                                        ssum4[:, st:st + 1])
                        return kT_seg, v_seg

                    prev = None
                    for kh in range(KH):
                        cur = build_kv(kh)
                        if prev is not None:
                            attend(kh - 1, prev[0], prev[1], bias_seg, first=(idx == 0))
                        prev = cur
                    attend(KH - 1, prev[0], prev[1], bias_seg, first=(idx == 0))

                for sx in range(NSEG_X):
                    ssl = slice(sx * SEG, (sx + 1) * SEG)
                    bias_seg = ap1.tile([CH, SEG_CH, TQ], F32, tag="biasseg", name="bias_seg")
                    nc.sync.dma_start(bias_seg[:],
                                      bcr[:, sx * SEG_CH:(sx + 1) * SEG_CH, :])
                    for kh in range(KH):
                        kT_seg = ap1.tile([CH, HCH, SEG], F32R, tag="kseg", name="kT_seg")
                        nc.sync.dma_start(
                            kT_seg[:], d_kc[kh].rearrange("(hc p) s -> p hc s", p=CH)[:, :, ssl])
                        v_seg = ap1.tile([CH, SEG_CH, H], F32R, tag="vseg", name="v_seg")
                        nc.sync.dma_start(
                            v_seg[:], d_vc[kh, ssl, :].rearrange("(st p) hh -> p st hh", p=CH))
                        attend(kh, kT_seg, v_seg, bias_seg, first=False)

            # ---- normalize O in place, then attn_vec -> DRAM scratch ----
            with tc.tile_pool(name="avp", bufs=2) as avpool, \
                    tc.tile_pool(name="wav_s", bufs=8) as wavs:
                pa_stat = ps_st.tile([1, TQ], F32, tag="stat", name="pa_stat")
                for n_head in range(N):
                    rs = avpool.tile([1, TQ], F32R, tag="rsum", name="rs")
                    nc.vector.reciprocal(rs[:], sums[:, n_head, :])
                    bc = ps_bc.tile([CH, TQ], F32, tag="bc", name="bc")
                    nc.tensor.matmul(bc[:], ones[:1, :], rs[:], start=True, stop=True)
                    for hc in range(HCH):
                        nc.vector.tensor_tensor(Oall[:, n_head, hc, :],
                                                Oall[:, n_head, hc, :], bc[:], OP.mult)
                for dcq in range(DCH // 4):
                    wav_ts = []
                    for n_head in range(N):
                        wav_t = wavs.tile([CH, HCH, 4 * CH], F32R, tag="wavf", name="wav_t")
                        nc.sync.dma_start(
                            wav_t[:], d_wav[n_head]
                            .rearrange("(hc p) d2 -> p hc d2", p=CH)
                            [:, :, dcq * 4 * CH:(dcq + 1) * 4 * CH])
                        wav_ts.append(wav_t)
                    for dcl in range(4):
                        dc = dcq * 4 + dcl
                        avp = ps_lg.tile([CH, TQ], F32, tag="mm", name="avp")
                        i = 0
                        for n_head in range(N):
                            for hc in range(HCH):
                                nc.tensor.matmul(avp[:],
                                                 wav_ts[n_head][:, hc, dcl * CH:(dcl + 1) * CH],
                                                 Oall[:, n_head, hc, :],
                                                 start=(i == 0), stop=(i == N * HCH - 1))
                                i += 1
                        avs = avpool.tile([CH, TQ], F32, tag="avs", name="avs")
                        nc.scalar.activation(out=avs[:], in_=avp[:], func=AF.Copy, scale=1.0)
                        nc.sync.dma_start(d_scr[dc], avs[:])
                        sqa = avpool.tile([CH, TQ], F32R, tag="sqa", name="sqa")
                        nc.scalar.activation(out=sqa[:], in_=avs[:], func=AF.Square, scale=1.0)
                        nc.tensor.matmul(pa_stat[:], ones[:, :1], sqa[:],
                                         start=(dc == 0), stop=(dc == DCH - 1))

        # ==================================================================
        # post-attn norm + residual; FFN
        # ==================================================================
        with tc.tile_pool(name="cdpool", bufs=1) as cd:
            attn_out = cd.tile([CH, DCH, TQ], F32)
            ffw_in = cd.tile([CH, DCH, TQ], F32R)
            with tc.tile_pool(name="phc", bufs=1) as pc1, \
                    tc.tile_pool(name="phc2", bufs=2) as pc2:
                x_own = pc1.tile([CH, DCH, TQ], F32R, tag="xown")
                nc.sync.dma_start(x_own[:], xr[:, :, OWN_OFF:OWN_OFF + TQ])
                acc = pc1.tile([CH, DCH, TQ], F32, tag="accrd")
                for dc in range(DCH):
                    nc.sync.dma_start(acc[:, dc, :], d_scr[dc])
                std = pc2.tile([1, TQ], F32, tag="std_pa", name="std")
                nc.scalar.activation(out=std[:], in_=pa_stat[:], func=AF.Sqrt,
                                     bias=eps1[:], scale=1.0 / D)
                rstd = pc2.tile([1, TQ], F32R, tag="rstd_pa", name="rstd")
                nc.vector.reciprocal(rstd[:], std[:])
                bcp = ps_bc.tile([CH, TQ], F32, tag="bc", name="bcp")
                nc.tensor.matmul(bcp[:], ones[:1, :], rstd[:], start=True, stop=True)
                bc = pc2.tile([CH, TQ], F32, tag="bc_pa", name="bc")
                nc.scalar.activation(out=bc[:], in_=bcp[:], func=AF.Copy, scale=1.0)
                for dc in range(DCH):
                    tt = pc2.tile([CH, TQ], F32, tag="catmp", name="tt")
                    nc.vector.tensor_tensor(tt[:], acc[:, dc, :], bc[:], OP.mult)
                    nc.vector.tensor_scalar_mul(tt[:], tt[:], pa_t[:, dc:dc + 1])
                    nc.vector.tensor_scalar_mul(attn_out[:, dc, :], x_own[:, dc, :], skip)
                    nc.vector.tensor_tensor(attn_out[:, dc, :], attn_out[:, dc, :], tt[:], OP.add)
                bc2 = stat_rstd(pc2, [attn_out[:, dc, :] for dc in range(DCH)], 1.0 / D, "pf")
                for dc in range(DCH):
                    nc.vector.tensor_tensor(ffw_in[:, dc, :], attn_out[:, dc, :], bc2[:], OP.mult)

            with tc.tile_pool(name="dp1", bufs=1) as dp1, \
                    tc.tile_pool(name="dp2", bufs=2) as dp2, \
                    tc.tile_pool(name="wg_s", bufs=4) as wgs, \
                    tc.tile_pool(name="wl_s", bufs=2) as wls:
                ffw_acc = dp1.tile([CH, DCH, TQ], F32)
                for fg in range(FGROUPS):
                    act = dp1.tile([CH, FPG, TQ], F32R, tag="act", name="act")
                    for fc in range(FPG):
                        f = fg * FPG + fc
                        gp = []
                        for g01 in range(2):
                            wg_t = wgs.tile([CH, DCH, CH], F32R, tag="wg", name="wg_t")
                            nc.sync.dma_start(wg_t[:], wgr[g01][:, :, f * CH:(f + 1) * CH])
                            pg = ps_kv.tile([CH, TQ], F32, tag="mm", name=f"pg{g01}")
                            for dc in range(DCH):
                                nc.tensor.matmul(pg[:], wg_t[:, dc, :], ffw_in[:, dc, :],
                                                 start=(dc == 0), stop=(dc == DCH - 1))
                            gp.append(pg)
                        gel = dp2.tile([CH, TQ], F32, tag="gel", name="gel")
                        nc.scalar.activation(out=gel[:], in_=gp[0][:],
                                             func=AF.Gelu_apprx_tanh, scale=1.0)
                        nc.vector.tensor_tensor(act[:, fc, :], gel[:], gp[1][:], OP.mult)
                    for dc in range(DCH):
                        wl_t = wls.tile([CH, FPG, CH], F32R, tag="wl", name="wl_t")
                        nc.sync.dma_start(
                            wl_t[:], wlr[:, fg * FPG:(fg + 1) * FPG,
                                         dc * CH:(dc + 1) * CH])
                        pf = ps_lg.tile([CH, TQ], F32, tag="mm", name="pf")
                        for fc in range(FPG):
                            nc.tensor.matmul(pf[:], wl_t[:, fc, :], act[:, fc, :],
                                             start=(fc == 0), stop=(fc == FPG - 1))
                        if fg == 0:
                            nc.scalar.activation(out=ffw_acc[:, dc, :], in_=pf[:],
                                                 func=AF.Copy, scale=1.0)
                        else:
                            nc.vector.tensor_tensor(ffw_acc[:, dc, :], ffw_acc[:, dc, :],
                                                    pf[:], OP.add)
                # post-ffw norm + final residual
                bc = stat_rstd(dp2, [ffw_acc[:, dc, :] for dc in range(DCH)], 1.0 / D, "of")
                for dc in range(DCH):
                    tt = dp2.tile([CH, TQ], F32, tag="fftmp", name="tt")
                    nc.vector.tensor_tensor(tt[:], ffw_acc[:, dc, :], bc[:], OP.mult)
                    nc.vector.tensor_scalar_mul(tt[:], tt[:], pf_t[:, dc:dc + 1])
                    ot = dp2.tile([CH, TQ], F32, tag="outt", name="ot")
                    nc.vector.tensor_tensor(ot[:], attn_out[:, dc, :], tt[:], OP.add)
                    nc.sync.dma_start(outr[:, dc, :], ot[:])

    nc.finalize()
    return nc


_NC_CACHE = {}


def kernel(**inputs) -> np.ndarray:
    from concourse.bass_utils import run_bass_kernel_spmd

    plan = _plan(inputs)
    w = _prep_weights(inputs)
    key = (plan['W'], plan['OWN_OFF'], plan['EXTRA'], w['skip'])
    if key not in _NC_CACHE:
        _NC_CACHE[key] = _build_nc(*key)
    nc = _NC_CACHE[key]

    ones = np.ones((CH, CH), np.float32)
    in_maps = []
    for c in range(N_CORES):
        e = plan['per_core'][c]
        m = dict(x_T=e['x_T'], sin_t=e['sin'], cos_t=e['cos'], bias=e['bias'],
                 w_q=w['w_q'], w_k=w['w_k'], w_v=w['w_v'], w_av=w['w_av'],
                 w_g_T=w['w_g_T'], w_lin=w['w_lin'],
                 q_scale=w['q_scale'], k_scale=w['k_scale'],
                 post_attn=w['post_attn'], post_ffw=w['post_ffw'], onesd=ones)
        if plan['EXTRA']:
            m.update(kc_T=e['kc_T'], vc=e['vc'], bias_cache=e['bias_cache'])
        in_maps.append(m)

    res = run_bass_kernel_spmd(nc, in_maps, core_ids=list(range(N_CORES)))

    out = np.zeros((B, T, D), np.float32)
    for c in range(N_CORES):
        e = plan['per_core'][c]
        out[e['b'], e['t0']:e['t0'] + TQ] = res.results[c]['out_T'].T
    return out

